# revision 16
# baseline (speedup 1.0000x reference)
"""PaDiM AnomalyMapGenerator kernel for 8 TRN2 NeuronCores.

Pipeline (per the reference):
  1. delta = embedding - mean                                   [B, C, HW]
  2. q[n, b] = delta_nb^T @ inv_cov_n @ delta_nb  (per pixel)   [HW, B]
  3. dist = sqrt(q) -> score maps                               [B, 56, 56]
  4. bilinear 4x upsample + 33x33 gaussian blur (reflect)       [B, 1, 224, 224]

Stage 4 is a fixed linear operator, so it collapses to out_b = M @ S_b @ M^T
with M = Blur(224x224, reflect) @ Resize(224x56) precomputed on host.

Sharding: pixels (HW) are split across the 8 cores for stages 1-3.  The
symmetric inv_covariance is shipped as a bf16 upper trapezoid in three row
blocks {64, 64, 72} packed partition-major on host, so each core streams
~21 MB in 14 ~1.5 MB DMAs with no on-chip casting.  The symmetry 2x factor
is folded into the data: the delta weights are pre-doubled and the diagonal
sub-blocks of inv_cov are pre-halved, so q = sum_j md_j * d_j comes out
exact with uniform matmuls.  delta is shipped pre-centered in the two
layouts the kernel needs (channel-major matmul weights, batch-major dot
operand), so the only on-chip work is matmul + one fused
multiply-accumulate DVE pass per pixel group + sqrt.

The tiny per-core score maps are AllGathered in two chunks (the first one
overlaps the second half of the main loop), then stage 4 is sharded by
OUTPUT ROWS: core j computes rows [28j, 28j+28) of every image.
"""

import os
import sys

import numpy as np

for _p in ("/opt/trn_rl_repo", "/root/.axon_site/_ro/trn_rl_repo"):
    if os.path.isdir(_p) and _p not in sys.path:
        sys.path.append(_p)

import ml_dtypes

import concourse.bacc as bacc
import concourse.mybir as mybir
import concourse.tile as tile
from concourse.bass_utils import run_bass_kernel_spmd

BF16NP = ml_dtypes.bfloat16

F32 = mybir.dt.float32
BF16 = mybir.dt.bfloat16

B, C, H, W = 32, 200, 56, 56
HW = H * W                 # 3136
NCORES = 8
NL = HW // NCORES          # 392 pixels per core
IMG = 224
SIGMA = 4.0
KS = 33
PAD = (KS - 1) // 2
RPC = IMG // NCORES        # output rows per core (post-proc row sharding)

# channel row-blocks of the inv_cov upper trapezoid
KBLK = (128, 72)           # rows 0:128, 128:200
KA, K2 = KBLK

GRP = 4                    # pixels per matmul group (PE column tiling)
NGRP = NL // GRP           # 98 groups
GPC = 7                    # groups per icov DMA chunk (28 pixels)
NCHUNK = NGRP // GPC       # 14 chunks


def _mt_matrix():
    """M^T [56, 224] for out_b = M @ S_b @ M^T == blur(upsample(S_b))."""
    scale = IMG / H
    u = (np.arange(IMG, dtype=np.float64) + 0.5) / scale - 0.5
    k = np.arange(H, dtype=np.float64)
    wts = np.maximum(0.0, 1.0 - np.abs(u[:, None] - k[None, :]))
    R = wts / wts.sum(axis=1, keepdims=True)          # [224, 56]
    x = np.arange(KS, dtype=np.float64) - (KS - 1) / 2.0
    g = np.exp(-(x * x) / (2.0 * SIGMA * SIGMA))
    g = g / g.sum()
    Bm = np.zeros((IMG, IMG), dtype=np.float64)
    for i in range(IMG):
        for j in range(KS):
            t = i + j - PAD
            if t < 0:
                t = -t
            if t >= IMG:
                t = 2 * IMG - 2 - t
            Bm[i, t] += g[j]
    M = Bm @ R                                         # [224, 56]
    return np.ascontiguousarray(M.T, dtype=np.float32)  # [56, 224]


def build():
    nc = bacc.Bacc("TRN2", target_bir_lowering=False, debug=False,
                   num_devices=NCORES)
    # w2 = 2*delta channel-major (matmul weights); d1 = delta batch-major
    w2 = nc.dram_tensor("w2", [C, B, NL], BF16, kind="ExternalInput").ap()
    d1 = nc.dram_tensor("d1", [128, NCHUNK, GPC, C], BF16,
                        kind="ExternalInput").ap()
    ica = nc.dram_tensor("ica", [KA, NL, C], BF16,
                         kind="ExternalInput").ap()
    icc = nc.dram_tensor("icc", [K2, NL, K2], BF16, kind="ExternalInput").ap()
    mt = nc.dram_tensor("mt", [H, IMG], F32, kind="ExternalInput").ap()
    mtj = nc.dram_tensor("mtj", [H, RPC], F32, kind="ExternalInput").ap()
    outp = nc.dram_tensor("out", [B, RPC, IMG], F32, kind="ExternalOutput").ap()

    with tile.TileContext(nc) as tc:
        with (
            tc.tile_pool(name="const", bufs=1) as cpool,
            tc.tile_pool(name="ic", bufs=3) as icpool,
            tc.tile_pool(name="d1p", bufs=3) as d1pool,
            tc.tile_pool(name="scr", bufs=4) as scrpool,
            tc.tile_pool(name="post", bufs=3) as postpool,
            tc.tile_pool(name="psmd", bufs=4, space="PSUM") as psmd,
            tc.tile_pool(name="psw", bufs=2, space="PSUM") as pswpool,
            tc.tile_pool(name="pso", bufs=2, space="PSUM") as psopool,
            tc.tile_pool(name="dram", bufs=1, space="DRAM") as dram,
        ):
            # ---- one-time loads (scalar=ACT HWDGE ring; sync streams icov) --
            # weight channel blocks {64, 64, 72} each live in their own tile
            # at base partition 0: matmul requires lhsT and rhs to start at
            # the same partition index.
            W2 = [cpool.tile([k, B, NL], BF16, tag=f"W2_{i}", name=f"W2_{i}")
                  for i, k in enumerate(KBLK)]
            boot_in = dram.tile([1, 16], F32, tag="boot_in")
            boot_out = dram.tile([NCORES, 16], F32, tag="boot_out")
            mts = cpool.tile([H, IMG], F32, tag="mts")
            mtsbf = cpool.tile([H, IMG], BF16, tag="mtsbf")
            mtjs = cpool.tile([H, RPC], F32, tag="mtjs")
            mtjbf = cpool.tile([H, RPC], BF16, tag="mtjbf")
            # per-half q tiles: keeps the first AllGather's sqrt free of
            # (whole-tile) dependencies on second-half writers
            QA = cpool.tile([128, NGRP // 2], F32, tag="QA")
            QB = cpool.tile([128, NGRP // 2], F32, tag="QB")
            QsA = cpool.tile([128, NGRP // 2], F32, tag="QsA")
            QsB = cpool.tile([128, NGRP // 2], F32, tag="QsB")

            # dummy collective: absorbs the one-time CC bootstrap barrier
            # (~50 us) under the start of the main loop
            nc.gpsimd.collective_compute(
                "AllGather", mybir.AluOpType.bypass,
                replica_groups=[list(range(NCORES))],
                ins=[boot_in[:].opt()],
                outs=[boot_out[:].opt()],
            )
            w2f = w2.rearrange("c b n -> c (b n)")
            for i, (ca, cb) in enumerate(((0, KA), (KA, C))):
                nc.scalar.dma_start(W2[i][:].rearrange("p b n -> p (b n)"),
                                    w2f[ca:cb])
            nc.scalar.dma_start(mts[:], mt)
            nc.scalar.dma_start(mtjs[:], mtj)
            nc.vector.tensor_copy(mtsbf[:], mts[:])
            nc.vector.tensor_copy(mtjbf[:], mtjs[:])


            # ---- per-pixel mahalanobis -------------------------------------
            # packed pixel m = 4g + p covers original pixel n = 98p + g.
            for ch in range(NCHUNK):
                icAt = icpool.tile([KA, GRP * GPC, C], BF16, tag="ica")
                icCt = icpool.tile([K2, GRP * GPC, K2], BF16, tag="icc")
                m0 = ch * GRP * GPC
                nc.sync.dma_start(
                    icAt[:].rearrange("p x y -> p (x y)"),
                    ica.rearrange("p n y -> p (n y)")
                    [:, m0 * C:(m0 + GRP * GPC) * C])
                nc.sync.dma_start(
                    icCt[:].rearrange("p x y -> p (x y)"),
                    icc.rearrange("p n y -> p (n y)")
                    [:, m0 * K2:(m0 + GRP * GPC) * K2])
                D1t = d1pool.tile([128, GPC, C], BF16, tag="d1t")
                nc.scalar.dma_start(
                    D1t[:].rearrange("p x y -> p (x y)"),
                    d1.rearrange("p ch x y -> p ch (x y)")[:, ch])

                for j in range(GPC):
                    g = GPC * ch + j
                    ps_md = psmd.tile([128, 512], F32, tag="psmd")
                    # md = (2 delta)^T @ icov-trapezoid (diags pre-halved);
                    # emitted block-major so consecutive matmuls hit
                    # different PE column groups and overlap
                    for p in range(GRP):
                        nc.tensor.matmul(
                            ps_md[32 * p:32 * p + 32, 0:C],
                            W2[0][:, :, m0 + GRP * j + p],
                            icAt[:, GRP * j + p, :],
                            start=True, stop=False,
                            tile_position=(0, 32 * p))
                    for p in range(GRP):
                        nc.tensor.matmul(
                            ps_md[32 * p:32 * p + 32, KA:C],
                            W2[1][:, :, m0 + GRP * j + p],
                            icCt[:, GRP * j + p, :],
                            start=False, stop=True,
                            tile_position=(0, 32 * p))
                    # fused q[:, g] = sum_c md * delta in one DVE pass
                    scr = scrpool.tile([128, C], F32, tag="scr")
                    Qh, gh = (QA, g) if g < NGRP // 2 else (QB, g - NGRP // 2)
                    nc.vector.scalar_tensor_tensor(
                        scr[:], ps_md[:, 0:C], 1.0, D1t[:, j, :],
                        op0=mybir.AluOpType.mult,
                        op1=mybir.AluOpType.mult,
                        accum_out=Qh[:, gh:gh + 1])

            # ---- dist = sqrt(q); relayout; two overlapped AllGathers -------
            HG = NGRP // 2                 # 49 groups per collective chunk
            dstA = cpool.tile([B, GRP, HG], F32, tag="dstA")
            dstB = cpool.tile([B, GRP, HG], F32, tag="dstB")
            dramA = dram.tile([B, GRP * HG], F32, tag="dramA")
            dramB = dram.tile([B, GRP * HG], F32, tag="dramB")
            gallA = dram.tile([NCORES * B, GRP * HG], F32, tag="gallA")
            gallB = dram.tile([NCORES * B, GRP * HG], F32, tag="gallB")
            s_dram = dram.tile([B, HW], F32, tag="s_dram")

            for half, (dst, drm, gall, Qh, Qsh) in enumerate(
                    [(dstA, dramA, gallA, QA, QsA),
                     (dstB, dramB, gallB, QB, QsB)]):
                gg = half * HG
                nc.scalar.sqrt(Qsh[:], Qh[:])
                for p in range(GRP):
                    nc.scalar.dma_start(dst[:, p, :],
                                        Qsh[32 * p:32 * p + 32, :])
                nc.scalar.dma_start(drm[:],
                                    dst[:].rearrange("b p g -> b (p g)"))
                nc.gpsimd.collective_compute(
                    "AllGather", mybir.AluOpType.bypass,
                    replica_groups=[list(range(NCORES))],
                    ins=[drm[:].opt()],
                    outs=[gall[:].opt()],
                )
                # s_dram[b, 392*sc + 98*p + g] = gall[32*sc + b, 49*p + g]
                sdv = s_dram[:].rearrange("b (sc p g) -> b sc p g",
                                          sc=NCORES, p=GRP)
                glv = gall[:].rearrange("(sc b) (p g) -> b sc p g", b=B, g=HG)
                for p in range(GRP):
                    nc.gpsimd.dma_start(sdv[:, :, p, gg:gg + HG],
                                        glv[:, :, p, :])

            # ---- post-proc: rows [28j, 28j+28) of M @ S_b @ M^T ------------
            sk_f = cpool.tile([H, B, W], F32, tag="sk_f")
            sk_all = cpool.tile([H, B, W], BF16, tag="sk_all")
            nc.gpsimd.dma_start(sk_f[:],
                                s_dram[:].rearrange("b (r c) -> r b c", c=W))
            nc.vector.tensor_copy(sk_all[:].rearrange("p b c -> p (b c)"),
                                  sk_f[:].rearrange("p b c -> p (b c)"))
            for t0 in range(0, B, 4):
                psw = pswpool.tile([H, 512], F32, tag="psw")
                for t in range(4):
                    nc.tensor.matmul(psw[:, 128 * t:128 * t + RPC],
                                     sk_all[:, t0 + t, :], mtjbf[:],
                                     start=(t == 0), stop=(t == 3))
                wsb = postpool.tile([H, 4, RPC], BF16, tag="wsb")
                nc.scalar.copy(
                    wsb[:],
                    psw[:].rearrange("p (x y) -> p x y", x=4)[:, :, 0:RPC])
                pso = psopool.tile([4 * RPC, 512], F32, tag="pso")
                nc.tensor.matmul(pso[:, 0:IMG],
                                 wsb[:].rearrange("p x y -> p (x y)"),
                                 mtsbf[:], start=True, stop=True)
                osb = postpool.tile([4 * RPC, IMG], F32, tag="osb")
                nc.vector.tensor_copy(osb[:], pso[:, 0:IMG])
                nc.sync.dma_start(
                    outp[t0:t0 + 4].rearrange("t i j -> (t i) j"), osb[:])

    nc.compile()
    return nc


_NC = None


def _get_nc():
    global _NC
    if _NC is None:
        _NC = build()
    return _NC


def _reorder_pixels(x):
    """Reorder the trailing pixel axis n = 98p + g  ->  m = 4g + p."""
    s = x.shape[:-1]
    return np.ascontiguousarray(
        x.reshape(*s, GRP, NGRP).swapaxes(-2, -1).reshape(*s, NL))


def make_in_maps(embedding, mean, inv_covariance):
    emb = np.asarray(embedding, dtype=np.float32).reshape(B, C, HW)
    mean = np.asarray(mean, dtype=np.float32)
    icov = np.asarray(inv_covariance, dtype=np.float32)
    mt = _mt_matrix()
    in_maps = []
    for i in range(NCORES):
        sl = slice(i * NL, (i + 1) * NL)
        delta = emb[:, :, sl] - mean[None, :, sl]         # [B, C, NL] f32
        # pre-doubled channel-major weights + batch-major dot operand
        w2 = _reorder_pixels((2.0 * delta).transpose(1, 0, 2)).astype(BF16NP)
        # d1[(p,b), chunk, j, c] = delta[b, c, 98p + (7*chunk + j-th group)]
        d1 = _reorder_pixels(delta.transpose(1, 0, 2))    # [C, B, NL(m)]
        d1 = np.ascontiguousarray(
            d1.reshape(C, B, NCHUNK, GPC, GRP).transpose(4, 1, 2, 3, 0)
            .reshape(128, NCHUNK, GPC, C)).astype(BF16NP)
        # bf16 upper trapezoid of the symmetric inv_cov, diag blocks halved,
        # packed partition-major: ica[k, m, :] = S_m[k, :] (cols<128 halved),
        # icc[k, m, :] = 0.5 * S_m[128+k, 128:200]
        slab = icov[sl][_reorder_pixels(np.arange(NL))]   # [NL, C, C]
        t0 = np.ascontiguousarray(slab[:, 0:KA, :].transpose(1, 0, 2))
        t0[:, :, 0:KA] *= 0.5
        icap = t0.astype(BF16NP)
        iccp = (0.5 * slab[:, 128:C, 128:C].transpose(1, 0, 2)).astype(BF16NP)
        in_maps.append({
            "w2": w2,
            "d1": d1,
            "ica": np.ascontiguousarray(icap),
            "icc": np.ascontiguousarray(iccp),
            "mt": mt,
            "mtj": np.ascontiguousarray(mt[:, i * RPC:(i + 1) * RPC]),
        })
    return in_maps


def run(embedding, mean, inv_covariance, trace=False):
    nc = _get_nc()
    in_maps = make_in_maps(embedding, mean, inv_covariance)
    res = run_bass_kernel_spmd(nc, in_maps, list(range(NCORES)), trace=trace)
    # core i returns out rows [28i, 28i+28) for all images
    full = np.concatenate([res.results[i]["out"] for i in range(NCORES)],
                          axis=1).reshape(B, 1, IMG, IMG)
    return np.ascontiguousarray(full, dtype=np.float32), res


def kernel(embedding, mean, inv_covariance, covariance=None):
    out, _ = run(embedding, mean, inv_covariance, trace=False)
    return out


# revision 18
# speedup vs baseline: 1.5214x; 1.5214x over previous
"""PaDiM AnomalyMapGenerator kernel for 8 TRN2 NeuronCores.

Pipeline (per the reference):
  1. delta = embedding - mean                                   [B, C, HW]
  2. q[n, b] = delta_nb^T @ inv_cov_n @ delta_nb  (per pixel)   [HW, B]
  3. dist = sqrt(q) -> score maps                               [B, 56, 56]
  4. bilinear 4x upsample + 33x33 gaussian blur (reflect)       [B, 1, 224, 224]

Stage 4 is a fixed linear operator, so it collapses to out_b = M @ S_b @ M^T
with M = Blur(224x224, reflect) @ Resize(224x56) precomputed on host.

Sharding: pixels (HW) are split across the 8 cores for stages 1-3.  The
symmetric inv_covariance is shipped as a bf16 upper trapezoid in three row
blocks {64, 64, 72} packed partition-major on host, so each core streams
~21 MB in 14 ~1.5 MB DMAs with no on-chip casting.  The symmetry 2x factor
is folded into the data: the delta weights are pre-doubled and the diagonal
sub-blocks of inv_cov are pre-halved, so q = sum_j md_j * d_j comes out
exact with uniform matmuls.  delta is shipped pre-centered in the two
layouts the kernel needs (channel-major matmul weights, batch-major dot
operand), so the only on-chip work is matmul + one fused
multiply-accumulate DVE pass per pixel group + sqrt.

The tiny per-core score maps are AllGathered in two chunks (the first one
overlaps the second half of the main loop), then stage 4 is sharded by
OUTPUT ROWS: core j computes rows [28j, 28j+28) of every image.
"""

import os
import sys

import numpy as np

for _p in ("/opt/trn_rl_repo", "/root/.axon_site/_ro/trn_rl_repo"):
    if os.path.isdir(_p) and _p not in sys.path:
        sys.path.append(_p)

import ml_dtypes

import concourse.bacc as bacc
import concourse.mybir as mybir
import concourse.tile as tile
from concourse.bass_utils import run_bass_kernel_spmd

BF16NP = ml_dtypes.bfloat16

F32 = mybir.dt.float32
BF16 = mybir.dt.bfloat16
FP8 = mybir.dt.float8e4
FP8NP = mybir.dt.np(mybir.dt.float8e4)

B, C, H, W = 32, 200, 56, 56
HW = H * W                 # 3136
NCORES = 8
NL = HW // NCORES          # 392 pixels per core
IMG = 224
SIGMA = 4.0
KS = 33
PAD = (KS - 1) // 2
RPC = IMG // NCORES        # output rows per core (post-proc row sharding)

# channel row-blocks of the inv_cov upper trapezoid
KBLK = (128, 72)           # rows 0:128, 128:200
KA, K2 = KBLK

GRP = 4                    # pixels per matmul group (PE column tiling)
NGRP = NL // GRP           # 98 groups
GPC = 7                    # groups per icov DMA chunk (28 pixels)
NCHUNK = NGRP // GPC       # 14 chunks


def _mt_matrix():
    """M^T [56, 224] for out_b = M @ S_b @ M^T == blur(upsample(S_b))."""
    scale = IMG / H
    u = (np.arange(IMG, dtype=np.float64) + 0.5) / scale - 0.5
    k = np.arange(H, dtype=np.float64)
    wts = np.maximum(0.0, 1.0 - np.abs(u[:, None] - k[None, :]))
    R = wts / wts.sum(axis=1, keepdims=True)          # [224, 56]
    x = np.arange(KS, dtype=np.float64) - (KS - 1) / 2.0
    g = np.exp(-(x * x) / (2.0 * SIGMA * SIGMA))
    g = g / g.sum()
    Bm = np.zeros((IMG, IMG), dtype=np.float64)
    for i in range(IMG):
        for j in range(KS):
            t = i + j - PAD
            if t < 0:
                t = -t
            if t >= IMG:
                t = 2 * IMG - 2 - t
            Bm[i, t] += g[j]
    M = Bm @ R                                         # [224, 56]
    return np.ascontiguousarray(M.T, dtype=np.float32)  # [56, 224]


def build():
    nc = bacc.Bacc("TRN2", target_bir_lowering=False, debug=False,
                   num_devices=NCORES)
    # w2 = 2*delta channel-major (matmul weights); d1 = delta batch-major
    w2 = nc.dram_tensor("w2", [C, B, NL], BF16, kind="ExternalInput").ap()
    d1 = nc.dram_tensor("d1", [128, NCHUNK, GPC, C], BF16,
                        kind="ExternalInput").ap()
    ica = nc.dram_tensor("ica", [KA, NL, C], FP8,
                         kind="ExternalInput").ap()
    icc = nc.dram_tensor("icc", [K2, NL, K2], FP8, kind="ExternalInput").ap()
    mt = nc.dram_tensor("mt", [H, IMG], F32, kind="ExternalInput").ap()
    mtj = nc.dram_tensor("mtj", [H, RPC], F32, kind="ExternalInput").ap()
    outp = nc.dram_tensor("out", [B, RPC, IMG], F32, kind="ExternalOutput").ap()

    with tile.TileContext(nc) as tc:
        with (
            tc.tile_pool(name="const", bufs=1) as cpool,
            tc.tile_pool(name="ic", bufs=3) as icpool,
            tc.tile_pool(name="d1p", bufs=3) as d1pool,
            tc.tile_pool(name="scr", bufs=4) as scrpool,
            tc.tile_pool(name="post", bufs=3) as postpool,
            tc.tile_pool(name="psmd", bufs=4, space="PSUM") as psmd,
            tc.tile_pool(name="psw", bufs=2, space="PSUM") as pswpool,
            tc.tile_pool(name="pso", bufs=2, space="PSUM") as psopool,
            tc.tile_pool(name="dram", bufs=1, space="DRAM") as dram,
        ):
            # ---- one-time loads (scalar=ACT HWDGE ring; sync streams icov) --
            # weight channel blocks {64, 64, 72} each live in their own tile
            # at base partition 0: matmul requires lhsT and rhs to start at
            # the same partition index.
            W2 = [cpool.tile([k, B, NL], BF16, tag=f"W2_{i}", name=f"W2_{i}")
                  for i, k in enumerate(KBLK)]
            boot_in = dram.tile([1, 16], F32, tag="boot_in")
            boot_out = dram.tile([NCORES, 16], F32, tag="boot_out")
            mts = cpool.tile([H, IMG], F32, tag="mts")
            mtsbf = cpool.tile([H, IMG], BF16, tag="mtsbf")
            mtjs = cpool.tile([H, RPC], F32, tag="mtjs")
            mtjbf = cpool.tile([H, RPC], BF16, tag="mtjbf")
            # per-half q tiles: keeps the first AllGather's sqrt free of
            # (whole-tile) dependencies on second-half writers
            QA = cpool.tile([128, NGRP // 2], F32, tag="QA")
            QB = cpool.tile([128, NGRP // 2], F32, tag="QB")
            QsA = cpool.tile([128, NGRP // 2], F32, tag="QsA")
            QsB = cpool.tile([128, NGRP // 2], F32, tag="QsB")

            # dummy collective: absorbs the one-time CC bootstrap barrier
            # (~50 us) under the start of the main loop
            nc.gpsimd.collective_compute(
                "AllGather", mybir.AluOpType.bypass,
                replica_groups=[list(range(NCORES))],
                ins=[boot_in[:].opt()],
                outs=[boot_out[:].opt()],
            )
            # W2 loads split by pixel halves so chunk-0 matmuls start early
            for h in range(2):
                n0, n1 = h * NL // 2, (h + 1) * NL // 2
                for i, (ca, cb) in enumerate(((0, KA), (KA, C))):
                    nc.scalar.dma_start(W2[i][:, :, n0:n1], w2[ca:cb, :,
                                                               n0:n1])
            nc.scalar.dma_start(mts[:], mt)
            nc.scalar.dma_start(mtjs[:], mtj)
            nc.vector.tensor_copy(mtsbf[:], mts[:])
            nc.vector.tensor_copy(mtjbf[:], mtjs[:])


            # ---- per-pixel mahalanobis -------------------------------------
            # packed pixel m = 4g + p covers original pixel n = 98p + g.
            for ch in range(NCHUNK):
                icAt = icpool.tile([KA, GRP * GPC, C], FP8, tag="ica")
                icCt = icpool.tile([K2, GRP * GPC, K2], FP8, tag="icc")
                m0 = ch * GRP * GPC
                nc.sync.dma_start(
                    icAt[:].rearrange("p x y -> p (x y)"),
                    ica.rearrange("p n y -> p (n y)")
                    [:, m0 * C:(m0 + GRP * GPC) * C])
                nc.sync.dma_start(
                    icCt[:].rearrange("p x y -> p (x y)"),
                    icc.rearrange("p n y -> p (n y)")
                    [:, m0 * K2:(m0 + GRP * GPC) * K2])
                D1t = d1pool.tile([128, GPC, C], BF16, tag="d1t")
                nc.sync.dma_start(
                    D1t[:].rearrange("p x y -> p (x y)"),
                    d1.rearrange("p ch x y -> p ch (x y)")[:, ch])

                for j in range(GPC):
                    g = GPC * ch + j
                    ps_md = psmd.tile([128, 512], F32, tag="psmd")
                    # md = (2 delta)^T @ icov-trapezoid (diags pre-halved);
                    # emitted block-major so consecutive matmuls hit
                    # different PE column groups and overlap
                    for p in range(GRP):
                        nc.tensor.matmul(
                            ps_md[32 * p:32 * p + 32, 0:C],
                            W2[0][:, :, m0 + GRP * j + p],
                            icAt[:, GRP * j + p, :],
                            start=True, stop=False,
                            tile_position=(0, 32 * p))
                    for p in range(GRP):
                        nc.tensor.matmul(
                            ps_md[32 * p:32 * p + 32, KA:C],
                            W2[1][:, :, m0 + GRP * j + p],
                            icCt[:, GRP * j + p, :],
                            start=False, stop=True,
                            tile_position=(0, 32 * p))
                    # fused q[:, g] = sum_c md * delta in one DVE pass
                    scr = scrpool.tile([128, C], F32, tag="scr")
                    Qh, gh = (QA, g) if g < NGRP // 2 else (QB, g - NGRP // 2)
                    nc.vector.scalar_tensor_tensor(
                        scr[:], ps_md[:, 0:C], 1.0, D1t[:, j, :],
                        op0=mybir.AluOpType.mult,
                        op1=mybir.AluOpType.mult,
                        accum_out=Qh[:, gh:gh + 1])

            # ---- dist = sqrt(q); relayout; two overlapped AllGathers -------
            HG = NGRP // 2                 # 49 groups per collective chunk
            dstA = cpool.tile([B, GRP, HG], F32, tag="dstA")
            dstB = cpool.tile([B, GRP, HG], F32, tag="dstB")
            dramA = dram.tile([B, GRP * HG], F32, tag="dramA")
            dramB = dram.tile([B, GRP * HG], F32, tag="dramB")
            gallA = dram.tile([NCORES * B, GRP * HG], F32, tag="gallA")
            gallB = dram.tile([NCORES * B, GRP * HG], F32, tag="gallB")
            s_dram = dram.tile([B, HW], F32, tag="s_dram")

            for half, (dst, drm, gall, Qh, Qsh) in enumerate(
                    [(dstA, dramA, gallA, QA, QsA),
                     (dstB, dramB, gallB, QB, QsB)]):
                gg = half * HG
                nc.scalar.sqrt(Qsh[:], Qh[:])
                for p in range(GRP):
                    nc.scalar.dma_start(dst[:, p, :],
                                        Qsh[32 * p:32 * p + 32, :])
                nc.scalar.dma_start(drm[:],
                                    dst[:].rearrange("b p g -> b (p g)"))
                nc.gpsimd.collective_compute(
                    "AllGather", mybir.AluOpType.bypass,
                    replica_groups=[list(range(NCORES))],
                    ins=[drm[:].opt()],
                    outs=[gall[:].opt()],
                )
                # s_dram[b, 392*sc + 98*p + g] = gall[32*sc + b, 49*p + g]
                sdv = s_dram[:].rearrange("b (sc p g) -> b sc p g",
                                          sc=NCORES, p=GRP)
                glv = gall[:].rearrange("(sc b) (p g) -> b sc p g", b=B, g=HG)
                for p in range(GRP):
                    nc.gpsimd.dma_start(sdv[:, :, p, gg:gg + HG],
                                        glv[:, :, p, :])

            # ---- post-proc: rows [28j, 28j+28) of M @ S_b @ M^T ------------
            sk_f = cpool.tile([H, B, W], F32, tag="sk_f")
            sk_all = cpool.tile([H, B, W], BF16, tag="sk_all")
            nc.gpsimd.dma_start(sk_f[:],
                                s_dram[:].rearrange("b (r c) -> r b c", c=W))
            nc.vector.tensor_copy(sk_all[:].rearrange("p b c -> p (b c)"),
                                  sk_f[:].rearrange("p b c -> p (b c)"))
            for t0 in range(0, B, 4):
                psw = pswpool.tile([H, 512], F32, tag="psw")
                for t in range(4):
                    nc.tensor.matmul(psw[:, 128 * t:128 * t + RPC],
                                     sk_all[:, t0 + t, :], mtjbf[:],
                                     start=(t == 0), stop=(t == 3))
                wsb = postpool.tile([H, 4, RPC], BF16, tag="wsb")
                nc.scalar.copy(
                    wsb[:],
                    psw[:].rearrange("p (x y) -> p x y", x=4)[:, :, 0:RPC])
                pso = psopool.tile([4 * RPC, 512], F32, tag="pso")
                nc.tensor.matmul(pso[:, 0:IMG],
                                 wsb[:].rearrange("p x y -> p (x y)"),
                                 mtsbf[:], start=True, stop=True)
                osb = postpool.tile([4 * RPC, IMG], F32, tag="osb")
                nc.vector.tensor_copy(osb[:], pso[:, 0:IMG])
                nc.sync.dma_start(
                    outp[t0:t0 + 4].rearrange("t i j -> (t i) j"), osb[:])

    nc.compile()
    return nc


_NC = None


def _get_nc():
    global _NC
    if _NC is None:
        _NC = build()
    return _NC


def _reorder_pixels(x):
    """Reorder the trailing pixel axis n = 98p + g  ->  m = 4g + p."""
    s = x.shape[:-1]
    return np.ascontiguousarray(
        x.reshape(*s, GRP, NGRP).swapaxes(-2, -1).reshape(*s, NL))


def make_in_maps(embedding, mean, inv_covariance):
    emb = np.asarray(embedding, dtype=np.float32).reshape(B, C, HW)
    mean = np.asarray(mean, dtype=np.float32)
    icov = np.asarray(inv_covariance, dtype=np.float32)
    mt = _mt_matrix()
    in_maps = []
    for i in range(NCORES):
        sl = slice(i * NL, (i + 1) * NL)
        delta = emb[:, :, sl] - mean[None, :, sl]         # [B, C, NL] f32
        # pre-doubled channel-major weights + batch-major dot operand
        w2 = _reorder_pixels((2.0 * delta).transpose(1, 0, 2)).astype(BF16NP)
        # d1[(p,b), chunk, j, c] = delta[b, c, 98p + (7*chunk + j-th group)]
        d1 = _reorder_pixels(delta.transpose(1, 0, 2))    # [C, B, NL(m)]
        d1 = np.ascontiguousarray(
            d1.reshape(C, B, NCHUNK, GPC, GRP).transpose(4, 1, 2, 3, 0)
            .reshape(128, NCHUNK, GPC, C)).astype(BF16NP)
        # bf16 upper trapezoid of the symmetric inv_cov, diag blocks halved,
        # packed partition-major: ica[k, m, :] = S_m[k, :] (cols<128 halved),
        # icc[k, m, :] = 0.5 * S_m[128+k, 128:200]
        slab = icov[sl][_reorder_pixels(np.arange(NL))]   # [NL, C, C]
        t0 = np.ascontiguousarray(slab[:, 0:KA, :].transpose(1, 0, 2))
        t0[:, :, 0:KA] *= 0.5
        icap = t0.astype(FP8NP)
        iccp = (0.5 * slab[:, 128:C, 128:C].transpose(1, 0, 2)).astype(FP8NP)
        in_maps.append({
            "w2": w2,
            "d1": d1,
            "ica": np.ascontiguousarray(icap),
            "icc": np.ascontiguousarray(iccp),
            "mt": mt,
            "mtj": np.ascontiguousarray(mt[:, i * RPC:(i + 1) * RPC]),
        })
    return in_maps


def run(embedding, mean, inv_covariance, trace=False):
    nc = _get_nc()
    in_maps = make_in_maps(embedding, mean, inv_covariance)
    res = run_bass_kernel_spmd(nc, in_maps, list(range(NCORES)), trace=trace)
    # core i returns out rows [28i, 28i+28) for all images
    full = np.concatenate([res.results[i]["out"] for i in range(NCORES)],
                          axis=1).reshape(B, 1, IMG, IMG)
    return np.ascontiguousarray(full, dtype=np.float32), res


def kernel(embedding, mean, inv_covariance, covariance=None):
    out, _ = run(embedding, mean, inv_covariance, trace=False)
    return out
